# revision 46
# baseline (speedup 1.0000x reference)
"""Trainium2 Bass kernel for nn_ActorHead (GNN edge-MLP with pairwise mean).

Strategy (8 NeuronCores, SPMD), v2:
  - Edges shared across B=4 batches: htab[n] = [h0[n]|h1[n]|h2[n]|h3[n]]
    (512 bf16 = 1024B rows); one gathered row serves all 4 batches.
  - Edge dim sharded across cores. On-device gather via dma_gather over 4
    SWDGE queues, int16 signed offsets from the middle of one of two
    65536-row windows; edges grouped host-side by (r-window, s-window).
  - Large gather ops (2048 idx) to amortize the ~1us fixed SWDGE cost;
    whole idx table DMAed once up front.
  - ~half of chunks gather in transpose mode (queue 0 only) landing
    directly [feature, batch, edge]; the rest are plain (queues 1-3) and
    transposed on the TensorEngine (identity matmul) + DVE copy (bitcast
    f32 to halve element count).
  - MLP bf16: y1 = relu(W1r^T hrT + W1s^T hsT + W1e^T eaT + b1) computed
    per 2-tile pairs (one PSUM [128,2,512] f32 tile, single relu op);
    y2 = W2^T y1 lagged one pair to keep PE from stalling on relu.
  - Output staged [2, B, 512] per 4 tiles, one DMA per 4 tiles.
  - Pairwise mean (+b2) on the host.
"""

import numpy as np
import ml_dtypes

B, N, E, EA = 4, 100000, 160000, 80000
HID, ED = 128, 16
NCORES = 8
WSZ = 65536                      # window size (signed-int16-addressable rows)
NW = (N + WSZ - 1) // WSZ        # 2 windows
WBASE = (32768, 65536 + (N - 65536) // 2)   # signed-offset base per window
OP_IDX = 1024                    # indices per dma_gather op
TILE = 128                       # edges per tile
PAIR = 2 * TILE                  # y1/relu granularity
QUAD = 4 * TILE                  # output-DMA granularity
TMODE_FRAC = 0.50                # fraction of chunks gathered in transpose mode

_cache = {}


def _wrap_idx(rel):
    n = len(rel)
    assert n % 16 == 0
    w = rel.reshape(n // 16, 16).T.astype(np.int16)
    return np.tile(w, (8, 1))


def _prepare(h, edge_index, edge_attr, edge_type_idx, W1, b1, W2, b2):
    bf16 = ml_dtypes.bfloat16
    sel = np.asarray(edge_index)[:, np.asarray(edge_type_idx)]
    sel_r = sel[0].astype(np.int64)
    sel_s = sel[1].astype(np.int64)

    wr = sel_r // WSZ
    ws = sel_s // WSZ
    gid = wr * NW + ws
    rel_r_all = sel_r - np.asarray(WBASE)[wr]
    rel_s_all = sel_s - np.asarray(WBASE)[ws]
    assert rel_r_all.min() >= -32768 and rel_r_all.max() <= 32767
    assert rel_s_all.min() >= -32768 and rel_s_all.max() <= 32767

    rng = np.random.default_rng(12345)
    group_edges = []
    for g in range(NW * NW):
        ge = np.nonzero(gid == g)[0]
        group_edges.append(rng.permutation(ge))
    S = []
    for g in range(NW * NW):
        per_core = -(-len(group_edges[g]) // NCORES)
        S.append(-(-max(per_core, 0) // PAIR) * PAIR if per_core else 0)
    NPAD = int(sum(S))

    ea_sel = np.asarray(edge_attr)[:, np.asarray(edge_type_idx), :]

    cores = []
    for c in range(NCORES):
        slot_edges = np.full(NPAD, -1, dtype=np.int64)
        idx_r = np.zeros(NPAD, dtype=np.int16)
        idx_s = np.zeros(NPAD, dtype=np.int16)
        off = 0
        for g in range(NW * NW):
            ge = group_edges[g]
            lo = (len(ge) * c) // NCORES
            hi = (len(ge) * (c + 1)) // NCORES
            part = ge[lo:hi]
            n = len(part)
            assert n <= S[g]
            slot_edges[off:off + n] = part
            idx_r[off:off + n] = rel_r_all[part].astype(np.int16)
            idx_s[off:off + n] = rel_s_all[part].astype(np.int16)
            # Guard: dma_gather strips TRAILING negative indices per op, so
            # the last slot of every OP_IDX chunk must be >=0 in BOTH
            # streams (pads are rel 0, fine). Swap in a qualifying edge.
            c0 = 0
            while c0 < S[g]:
                cn = min(S[g] - c0, OP_IDX)
                last = off + c0 + cn - 1
                if idx_r[last] < 0 or idx_s[last] < 0:
                    span = np.arange(off + c0, off + c0 + cn)
                    ok = np.nonzero((idx_r[span] >= 0) & (idx_s[span] >= 0))[0]
                    assert len(ok) > 0, "no nonneg-rel slot in gather op"
                    j = span[ok[0]]
                    for arr in (idx_r, idx_s, slot_edges):
                        arr[last], arr[j] = arr[j], arr[last]
                c0 += cn
            off += S[g]

        ea_pad = np.zeros((B, NPAD, ED), dtype=np.float32)
        valid = slot_edges >= 0
        ea_pad[:, valid, :] = ea_sel[:, slot_edges[valid], :]
        eat = ea_pad.reshape(B, NPAD // 128, 128, ED).transpose(3, 1, 0, 2)
        eat = np.ascontiguousarray(eat.reshape(ED, NPAD * B)).astype(bf16)

        idx_all = np.concatenate([_wrap_idx(idx_r), _wrap_idx(idx_s)], axis=1)
        cores.append({"slot_edges": slot_edges, "idx": idx_all, "eat": eat})

    h_np = np.asarray(h, dtype=np.float32)
    htab = np.ascontiguousarray(h_np.transpose(1, 0, 2).reshape(N, B * HID)).astype(bf16)

    W1_np = np.asarray(W1, dtype=np.float32)
    wts = {
        "w1r": np.ascontiguousarray(W1_np[:HID]).astype(bf16),
        "w1s": np.ascontiguousarray(W1_np[HID:2 * HID]).astype(bf16),
        "w1e": np.ascontiguousarray(W1_np[2 * HID:]).astype(bf16),
        "w2": np.ascontiguousarray(np.asarray(W2, dtype=np.float32)).astype(bf16),
        "b1": np.asarray(b1, dtype=np.float32).reshape(HID, 1).copy(),
        "ident": np.eye(128, dtype=np.float32).astype(bf16),
    }
    return htab, wts, cores, {"S": S, "NPAD": NPAD}


def _build(S, NPAD):
    import concourse.mybir as mybir
    from concourse import bacc
    from concourse.tile import TileContext

    bf = mybir.dt.bfloat16
    f32 = mybir.dt.float32

    nc = bacc.Bacc("TRN2", target_bir_lowering=False, debug=False,
                   num_devices=NCORES, num_swdge_queues=4,
                   dynamic_dma_scratch_size=32768)

    htab = nc.dram_tensor("htab", [N, B * HID], bf, kind="ExternalInput").ap()
    idx_ext = nc.dram_tensor("idx", [128, 2 * NPAD // 16], mybir.dt.int16,
                             kind="ExternalInput").ap()
    eat_ext = nc.dram_tensor("eat", [ED, NPAD * B], bf, kind="ExternalInput").ap()
    w1r_ext = nc.dram_tensor("w1r", [HID, HID], bf, kind="ExternalInput").ap()
    w1s_ext = nc.dram_tensor("w1s", [HID, HID], bf, kind="ExternalInput").ap()
    w1e_ext = nc.dram_tensor("w1e", [ED, HID], bf, kind="ExternalInput").ap()
    w2_ext = nc.dram_tensor("w2", [HID, 2], bf, kind="ExternalInput").ap()
    b1_ext = nc.dram_tensor("b1", [HID, 1], f32, kind="ExternalInput").ap()
    id_ext = nc.dram_tensor("ident", [128, 128], bf, kind="ExternalInput").ap()
    out_ext = nc.dram_tensor("out", [2, B, NPAD], f32, kind="ExternalOutput").ap()

    RELU = mybir.ActivationFunctionType.Relu

    with TileContext(nc) as tc:
        with (
            tc.tile_pool(name="const", bufs=1) as cp,
            tc.tile_pool(name="gr", bufs=3) as grp,
            tc.tile_pool(name="gs", bufs=3) as gsp,
            tc.tile_pool(name="eap", bufs=2) as eap,
            tc.tile_pool(name="tsb", bufs=3) as tsb,
            tc.tile_pool(name="y1s", bufs=2) as y1sp,
            tc.tile_pool(name="stg", bufs=2) as stgp,
            tc.tile_pool(name="ptp", bufs=2, space="PSUM") as ptp,
            tc.tile_pool(name="y1p", bufs=2, space="PSUM") as y1pp,
            tc.tile_pool(name="y2p", bufs=2, space="PSUM") as y2pp,
        ):
            idx_sb = cp.tile([128, 2 * NPAD // 16], mybir.dt.int16)
            nc.sync.dma_start(out=idx_sb[:], in_=idx_ext[:])
            w1r = cp.tile([HID, HID], bf)
            nc.sync.dma_start(out=w1r[:], in_=w1r_ext[:])
            w1s = cp.tile([HID, HID], bf)
            nc.sync.dma_start(out=w1s[:], in_=w1s_ext[:])
            w1e = cp.tile([ED, HID], bf)
            nc.sync.dma_start(out=w1e[:], in_=w1e_ext[:])
            w2 = cp.tile([HID, 2], bf)
            nc.sync.dma_start(out=w2[:], in_=w2_ext[:])
            b1 = cp.tile([HID, 1], f32)
            nc.sync.dma_start(out=b1[:], in_=b1_ext[:])
            ident = cp.tile([128, 128], bf)
            nc.sync.dma_start(out=ident[:], in_=id_ext[:])

            goffs = {}
            _off = 0
            for g in range(NW * NW):
                goffs[g] = _off
                _off += S[g]

            qc = 0          # plain-queue rotation
            tmc = [0, 0]    # Bresenham counters for tmode assignment (r, s)
            order = sorted((g for g in range(NW * NW) if S[g] > 0),
                           key=lambda g: S[g])

            # deferred-W2 pipeline state: list of pending (y1sb, ea-info...)
            pend = []       # entries: dict(y1sb, t0_global_cols, nt_pair)
            stg_cur = {"tile": None, "base": None, "filled": 0}

            def flush_stg():
                if stg_cur["tile"] is not None:
                    w = stg_cur["filled"]
                    nc.scalar.dma_start(
                        out=out_ext[:, :, stg_cur["base"]:stg_cur["base"] + w],
                        in_=stg_cur["tile"][:, :, :w],
                    )
                    stg_cur["tile"] = None

            def flush_w2():
                """Emit W2 + y2 copy + (maybe) out DMA for the oldest pending
                pair, if any."""
                if not pend:
                    return
                e = pend.pop(0)
                y1sb = e["y1sb"]
                for k in range(e["npair"]):
                    col0 = e["col0"] + k * TILE
                    y2 = y2pp.tile([2, B * HID], f32, tag="y2", space="PSUM")
                    nc.tensor.matmul(out=y2[:], lhsT=w2[:],
                                     rhs=y1sb[:, k, :],
                                     start=True, stop=True)
                    # stage into [2, B, 512] quad buffer
                    if (stg_cur["tile"] is None
                            or col0 != stg_cur["base"] + stg_cur["filled"]
                            or stg_cur["filled"] >= QUAD):
                        flush_stg()
                        stg_cur["tile"] = stgp.tile([2, B, QUAD], f32, tag="stg",
                                                    name="stg")
                        stg_cur["base"] = col0
                        stg_cur["filled"] = 0
                    o = col0 - stg_cur["base"]
                    nc.vector.tensor_copy(
                        out=stg_cur["tile"][:, :, o:o + TILE],
                        in_=y2[:].rearrange("p (b e) -> p b e", b=B),
                    )
                    stg_cur["filled"] = o + TILE

            for g in order:
                sg = S[g]
                goff = goffs[g]
                wrw = g // NW
                wsw = g % NW
                nt = sg // TILE

                ops = {"r": [], "s": []}
                ea_ops = []
                c0 = 0
                si = 0
                while c0 < nt:
                    cn = min(nt - c0, OP_IDX // TILE)
                    ni = cn * TILE
                    # phased modes [P,P,T,T]: plain chunks (Pool-cheap,
                    # PE-heavy) build PE backlog that the following tmode
                    # chunks (Pool-heavy, PE-cheap) let the PE drain --
                    # keeps the PE saturated instead of lockstepped
                    streams = (
                        ("r", wrw, 0, grp), ("s", wsw, NPAD // 16, gsp),
                    )
                    phase_tmode = (tmc[0] % 4) >= 2
                    tmc[0] += 1
                    for sk, win, stream_off, pool in streams:
                        whi = min(win * WSZ + WSZ, N)
                        wmid = WBASE[win]
                        tmode = phase_tmode
                        icol = stream_off + (goff + c0 * TILE) // 16
                        if tmode:
                            dst = pool.tile([128, B, ni], bf, tag=f"ht{sk}",
                                            name=f"ht{sk}")
                        else:
                            dst = pool.tile([128, cn, B * HID], bf,
                                            tag=f"hp{sk}", name=f"hp{sk}")
                        dview = dst[:]
                        nc.gpsimd.dma_gather(
                            out_ap=dview,
                            in_ap=htab[wmid:whi],
                            idxs_ap=idx_sb[:, icol:icol + ni // 16],
                            num_idxs=ni,
                            num_idxs_reg=ni,
                            elem_size=B * HID,
                            transpose=tmode,
                            single_packet=False,
                            queue_num=0 if tmode else 1 + qc % 3,
                        )
                        if not tmode:
                            qc += 1
                        ops[sk].append((dst, c0, cn, tmode))
                    # ea chunks at half gather-chunk size to save SBUF
                    e0 = c0
                    while e0 < c0 + cn:
                        en = min(c0 + cn - e0, OP_IDX // (2 * TILE))
                        ea_g = eap.tile([ED, en * B * TILE], bf, tag="ea",
                                        name="ea")
                        nc.sync.dma_start(
                            out=ea_g[:],
                            in_=eat_ext[:, B * (goff + e0 * TILE):
                                        B * (goff + (e0 + en) * TILE)])
                        ea_ops.append((ea_g, e0, en, False))
                        e0 += en
                    c0 += cn
                    si += 1

                def _op_slice(oplist, t):
                    for (dst, c0, cn, tm) in oplist:
                        if c0 <= t < c0 + cn:
                            return dst, t - c0, tm
                    raise AssertionError

                # compute loop: quads (2 pairs) of tiles for weight reuse
                npairs = nt // 2
                cpc = [0]  # copy-engine alternation counter

                def prep_rhs(t, rhs, k):
                    """Resolve rhs sources for tile t; emit transposes+copy
                    for plain-gathered streams (copies alternate DVE/ACT)."""
                    plain = []
                    for si_, sk in enumerate(("r", "s")):
                        d, l, tm = _op_slice(ops[sk], t)
                        if tm:
                            rhs[(k, sk)] = d[:, :, l * TILE:(l + 1) * TILE]
                        else:
                            plain.append((si_, sk, d, l))
                    if not plain:
                        return
                    pt = ptp.tile([128, 2, B * HID], bf,
                                  tag="pt", name="pt", space="PSUM")
                    hT = tsb.tile([128, 2, B * HID], bf, tag="hT", name="hT")
                    for (si_, sk, d, l) in plain:
                        for b in range(B):
                            nc.tensor.transpose(
                                out=pt[:, si_, b * HID:(b + 1) * HID],
                                in_=d[:, l, b * HID:(b + 1) * HID],
                                identity=ident[:],
                            )
                    if len(plain) == 2:
                        src = pt[:].bitcast(f32)
                        dst2 = hT[:].bitcast(f32)
                    else:
                        si_ = plain[0][0]
                        src = pt[:, si_, :].bitcast(f32)
                        dst2 = hT[:, si_, :].bitcast(f32)
                    nc.vector.tensor_copy(out=dst2, in_=src)
                    cpc[0] += 1
                    for (si_, sk, d, l) in plain:
                        rhs[(k, sk)] = hT[:, si_, :]

                p = 0
                while p < npairs:
                    np_ = min(1, npairs - p)   # pairs in this weight group
                    y1t = []
                    rhs = {}
                    for j in range(np_):
                        y1 = y1pp.tile([128, 2, B * HID], f32, tag="y1",
                                       name="y1", space="PSUM")
                        y1t.append(y1)
                        for k in range(2):
                            prep_rhs(2 * (p + j) + k, rhs, 2 * j + k)
                    # weight-grouped MLP across the quad (LDW every 4 MMs)
                    for w, skey, start, stop in (
                        (w1r, "r", True, False), (w1s, "s", False, False),
                    ):
                        for j in range(np_):
                            for k in range(2):
                                nc.tensor.matmul(out=y1t[j][:, k, :], lhsT=w[:],
                                                 rhs=rhs[(2 * j + k, skey)],
                                                 start=start, stop=stop)
                    for j in range(np_):
                        for k in range(2):
                            t = 2 * (p + j) + k
                            eg, le, _ = _op_slice(ea_ops, t)
                            nc.tensor.matmul(
                                out=y1t[j][:, k, :], lhsT=w1e[:],
                                rhs=eg[:, le * B * HID:(le + 1) * B * HID],
                                start=False, stop=True)
                    # lagged W2 keeps PE busy during relu
                    for _ in range(np_):
                        flush_w2()
                    for j in range(np_):
                        y1sb = y1sp.tile([128, 2, B * HID], bf, tag="y1sb",
                                         name="y1sb")
                        nc.scalar.activation(out=y1sb[:], in_=y1t[j][:],
                                             func=RELU, bias=b1[:])
                        pend.append({"y1sb": y1sb,
                                     "col0": goff + 2 * (p + j) * TILE,
                                     "npair": 2})
                    p += np_
            while pend:
                flush_w2()
            flush_stg()
    nc.compile()
    return nc


def _run(inputs, trace=False):
    import time as _t
    from concourse.bass_utils import run_bass_kernel_spmd

    htab, wts, cores, meta = _prepare(**inputs)
    key = tuple(meta["S"])
    if key not in _cache:
        t0 = _t.time()
        _cache[key] = _build(meta["S"], meta["NPAD"])
        print(f"[kernel] build+compile: {_t.time()-t0:.1f}s NPAD={meta['NPAD']}")
    nc = _cache[key]

    in_maps = []
    for c in range(NCORES):
        m = {"htab": htab, "eat": cores[c]["eat"], "idx": cores[c]["idx"]}
        m.update({k: wts[k] for k in ("w1r", "w1s", "w1e", "w2", "b1", "ident")})
        in_maps.append(m)

    res = run_bass_kernel_spmd(nc, in_maps, core_ids=list(range(NCORES)),
                               trace=trace)

    y2 = np.zeros((B, EA, 2), dtype=np.float32)
    for c in range(NCORES):
        o = res.results[c]["out"]
        se = cores[c]["slot_edges"]
        valid = se >= 0
        y2[:, se[valid], :] = o[:, :, valid].transpose(1, 2, 0)

    b2 = np.asarray(inputs["b2"], dtype=np.float32)
    out = 0.5 * (y2[:, 0::2, :] + y2[:, 1::2, :]) + b2[None, None, :]
    return out.astype(np.float32), res.exec_time_ns


def kernel(**inputs):
    out, _ = _run(inputs, trace=False)
    return out


# revision 47
# speedup vs baseline: 1.0928x; 1.0928x over previous
"""Trainium2 Bass kernel for nn_ActorHead (GNN edge-MLP with pairwise mean).

Strategy (8 NeuronCores, SPMD), v2:
  - Edges shared across B=4 batches: htab[n] = [h0[n]|h1[n]|h2[n]|h3[n]]
    (512 bf16 = 1024B rows); one gathered row serves all 4 batches.
  - Edge dim sharded across cores. On-device gather via dma_gather over 4
    SWDGE queues, int16 signed offsets from the middle of one of two
    65536-row windows; edges grouped host-side by (r-window, s-window).
  - Large gather ops (2048 idx) to amortize the ~1us fixed SWDGE cost;
    whole idx table DMAed once up front.
  - ~half of chunks gather in transpose mode (queue 0 only) landing
    directly [feature, batch, edge]; the rest are plain (queues 1-3) and
    transposed on the TensorEngine (identity matmul) + DVE copy (bitcast
    f32 to halve element count).
  - MLP bf16: y1 = relu(W1r^T hrT + W1s^T hsT + W1e^T eaT + b1) computed
    per 2-tile pairs (one PSUM [128,2,512] f32 tile, single relu op);
    y2 = W2^T y1 lagged one pair to keep PE from stalling on relu.
  - Output staged [2, B, 512] per 4 tiles, one DMA per 4 tiles.
  - Pairwise mean (+b2) on the host.
"""

import numpy as np
import ml_dtypes

B, N, E, EA = 4, 100000, 160000, 80000
HID, ED = 128, 16
NCORES = 8
WSZ = 65536                      # window size (signed-int16-addressable rows)
NW = (N + WSZ - 1) // WSZ        # 2 windows
WBASE = (32768, 65536 + (N - 65536) // 2)   # signed-offset base per window
OP_IDX = 1024                    # indices per dma_gather op
TILE = 128                       # edges per tile
PAIR = 2 * TILE                  # y1/relu granularity
QUAD = 4 * TILE                  # output-DMA granularity
TMODE_FRAC = 0.50                # fraction of chunks gathered in transpose mode

_cache = {}


def _wrap_idx(rel):
    n = len(rel)
    assert n % 16 == 0
    w = rel.reshape(n // 16, 16).T.astype(np.int16)
    return np.tile(w, (8, 1))


def _prepare(h, edge_index, edge_attr, edge_type_idx, W1, b1, W2, b2):
    bf16 = ml_dtypes.bfloat16
    sel = np.asarray(edge_index)[:, np.asarray(edge_type_idx)]
    sel_r = sel[0].astype(np.int64)
    sel_s = sel[1].astype(np.int64)

    wr = sel_r // WSZ
    ws = sel_s // WSZ
    gid = wr * NW + ws
    rel_r_all = sel_r - np.asarray(WBASE)[wr]
    rel_s_all = sel_s - np.asarray(WBASE)[ws]
    assert rel_r_all.min() >= -32768 and rel_r_all.max() <= 32767
    assert rel_s_all.min() >= -32768 and rel_s_all.max() <= 32767

    rng = np.random.default_rng(12345)
    group_edges = []
    for g in range(NW * NW):
        ge = np.nonzero(gid == g)[0]
        group_edges.append(rng.permutation(ge))
    S = []
    for g in range(NW * NW):
        per_core = -(-len(group_edges[g]) // NCORES)
        S.append(-(-max(per_core, 0) // PAIR) * PAIR if per_core else 0)
    NPAD = int(sum(S))

    ea_sel = np.asarray(edge_attr)[:, np.asarray(edge_type_idx), :]

    cores = []
    for c in range(NCORES):
        slot_edges = np.full(NPAD, -1, dtype=np.int64)
        idx_r = np.zeros(NPAD, dtype=np.int16)
        idx_s = np.zeros(NPAD, dtype=np.int16)
        off = 0
        for g in range(NW * NW):
            ge = group_edges[g]
            lo = (len(ge) * c) // NCORES
            hi = (len(ge) * (c + 1)) // NCORES
            part = ge[lo:hi]
            n = len(part)
            assert n <= S[g]
            slot_edges[off:off + n] = part
            idx_r[off:off + n] = rel_r_all[part].astype(np.int16)
            idx_s[off:off + n] = rel_s_all[part].astype(np.int16)
            # Guard: dma_gather strips TRAILING negative indices per op, so
            # the last slot of every OP_IDX chunk must be >=0 in BOTH
            # streams (pads are rel 0, fine). Swap in a qualifying edge.
            c0 = 0
            while c0 < S[g]:
                cn = min(S[g] - c0, OP_IDX)
                last = off + c0 + cn - 1
                if idx_r[last] < 0 or idx_s[last] < 0:
                    span = np.arange(off + c0, off + c0 + cn)
                    ok = np.nonzero((idx_r[span] >= 0) & (idx_s[span] >= 0))[0]
                    assert len(ok) > 0, "no nonneg-rel slot in gather op"
                    j = span[ok[0]]
                    for arr in (idx_r, idx_s, slot_edges):
                        arr[last], arr[j] = arr[j], arr[last]
                c0 += cn
            off += S[g]

        ea_pad = np.zeros((B, NPAD, ED), dtype=np.float32)
        valid = slot_edges >= 0
        ea_pad[:, valid, :] = ea_sel[:, slot_edges[valid], :]
        eat = ea_pad.reshape(B, NPAD // 128, 128, ED).transpose(3, 1, 0, 2)
        eat = np.ascontiguousarray(eat.reshape(ED, NPAD * B)).astype(bf16)

        idx_all = np.concatenate([_wrap_idx(idx_r), _wrap_idx(idx_s)], axis=1)
        cores.append({"slot_edges": slot_edges, "idx": idx_all, "eat": eat})

    h_np = np.asarray(h, dtype=np.float32)
    htab = np.ascontiguousarray(h_np.transpose(1, 0, 2).reshape(N, B * HID)).astype(bf16)

    W1_np = np.asarray(W1, dtype=np.float32)
    wts = {
        "w1r": np.ascontiguousarray(W1_np[:HID]).astype(bf16),
        "w1s": np.ascontiguousarray(W1_np[HID:2 * HID]).astype(bf16),
        "w1e": np.ascontiguousarray(W1_np[2 * HID:]).astype(bf16),
        "w2": np.ascontiguousarray(np.asarray(W2, dtype=np.float32)).astype(bf16),
        "b1": np.asarray(b1, dtype=np.float32).reshape(HID, 1).copy(),
        "ident": np.eye(128, dtype=np.float32).astype(bf16),
    }
    return htab, wts, cores, {"S": S, "NPAD": NPAD}


def _build(S, NPAD):
    import concourse.mybir as mybir
    from concourse import bacc
    from concourse.tile import TileContext

    bf = mybir.dt.bfloat16
    f32 = mybir.dt.float32

    nc = bacc.Bacc("TRN2", target_bir_lowering=False, debug=False,
                   num_devices=NCORES, num_swdge_queues=4,
                   dynamic_dma_scratch_size=32768)

    htab = nc.dram_tensor("htab", [N, B * HID], bf, kind="ExternalInput").ap()
    idx_ext = nc.dram_tensor("idx", [128, 2 * NPAD // 16], mybir.dt.int16,
                             kind="ExternalInput").ap()
    eat_ext = nc.dram_tensor("eat", [ED, NPAD * B], bf, kind="ExternalInput").ap()
    w1r_ext = nc.dram_tensor("w1r", [HID, HID], bf, kind="ExternalInput").ap()
    w1s_ext = nc.dram_tensor("w1s", [HID, HID], bf, kind="ExternalInput").ap()
    w1e_ext = nc.dram_tensor("w1e", [ED, HID], bf, kind="ExternalInput").ap()
    w2_ext = nc.dram_tensor("w2", [HID, 2], bf, kind="ExternalInput").ap()
    b1_ext = nc.dram_tensor("b1", [HID, 1], f32, kind="ExternalInput").ap()
    id_ext = nc.dram_tensor("ident", [128, 128], bf, kind="ExternalInput").ap()
    out_ext = nc.dram_tensor("out", [2, B, NPAD], f32, kind="ExternalOutput").ap()

    RELU = mybir.ActivationFunctionType.Relu

    with TileContext(nc) as tc:
        with (
            tc.tile_pool(name="const", bufs=1) as cp,
            tc.tile_pool(name="gr", bufs=2) as grp,
            tc.tile_pool(name="gs", bufs=2) as gsp,
            tc.tile_pool(name="eap", bufs=2) as eap,
            tc.tile_pool(name="tsb", bufs=3) as tsb,
            tc.tile_pool(name="y1s", bufs=2) as y1sp,
            tc.tile_pool(name="stg", bufs=2) as stgp,
            tc.tile_pool(name="ptp", bufs=2, space="PSUM") as ptp,
            tc.tile_pool(name="y1p", bufs=2, space="PSUM") as y1pp,
            tc.tile_pool(name="y2p", bufs=2, space="PSUM") as y2pp,
        ):
            idx_sb = cp.tile([128, 2 * NPAD // 16], mybir.dt.int16)
            nc.sync.dma_start(out=idx_sb[:], in_=idx_ext[:])
            w1r = cp.tile([HID, HID], bf)
            nc.sync.dma_start(out=w1r[:], in_=w1r_ext[:])
            w1s = cp.tile([HID, HID], bf)
            nc.sync.dma_start(out=w1s[:], in_=w1s_ext[:])
            w1e = cp.tile([ED, HID], bf)
            nc.sync.dma_start(out=w1e[:], in_=w1e_ext[:])
            w2 = cp.tile([HID, 2], bf)
            nc.sync.dma_start(out=w2[:], in_=w2_ext[:])
            b1 = cp.tile([HID, 1], f32)
            nc.sync.dma_start(out=b1[:], in_=b1_ext[:])
            ident = cp.tile([128, 128], bf)
            nc.sync.dma_start(out=ident[:], in_=id_ext[:])

            goffs = {}
            _off = 0
            for g in range(NW * NW):
                goffs[g] = _off
                _off += S[g]

            qc = 0          # plain-queue rotation
            tmc = [0, 0]    # Bresenham counters for tmode assignment (r, s)
            order = sorted((g for g in range(NW * NW) if S[g] > 0),
                           key=lambda g: S[g])

            # deferred-W2 pipeline state: list of pending (y1sb, ea-info...)
            pend = []       # entries: dict(y1sb, t0_global_cols, nt_pair)
            stg_cur = {"tile": None, "base": None, "filled": 0}

            def flush_stg():
                if stg_cur["tile"] is not None:
                    w = stg_cur["filled"]
                    nc.scalar.dma_start(
                        out=out_ext[:, :, stg_cur["base"]:stg_cur["base"] + w],
                        in_=stg_cur["tile"][:, :, :w],
                    )
                    stg_cur["tile"] = None

            def flush_w2():
                """Emit W2 + y2 copy + (maybe) out DMA for the oldest pending
                pair, if any."""
                if not pend:
                    return
                e = pend.pop(0)
                y1sb = e["y1sb"]
                for k in range(e["npair"]):
                    col0 = e["col0"] + k * TILE
                    y2 = y2pp.tile([2, B * HID], f32, tag="y2", space="PSUM")
                    nc.tensor.matmul(out=y2[:], lhsT=w2[:],
                                     rhs=y1sb[:, k, :],
                                     start=True, stop=True)
                    # stage into [2, B, 512] quad buffer
                    if (stg_cur["tile"] is None
                            or col0 != stg_cur["base"] + stg_cur["filled"]
                            or stg_cur["filled"] >= QUAD):
                        flush_stg()
                        stg_cur["tile"] = stgp.tile([2, B, QUAD], f32, tag="stg",
                                                    name="stg")
                        stg_cur["base"] = col0
                        stg_cur["filled"] = 0
                    o = col0 - stg_cur["base"]
                    nc.vector.tensor_copy(
                        out=stg_cur["tile"][:, :, o:o + TILE],
                        in_=y2[:].rearrange("p (b e) -> p b e", b=B),
                    )
                    stg_cur["filled"] = o + TILE

            for g in order:
                sg = S[g]
                goff = goffs[g]
                wrw = g // NW
                wsw = g % NW
                nt = sg // TILE

                ops = {"r": [], "s": []}
                ea_ops = []
                c0 = 0
                si = 0
                while c0 < nt:
                    cn = min(nt - c0, OP_IDX // TILE)
                    ni = cn * TILE
                    # alternate stream emission order so queue-0 (tmode) ops
                    # are evenly spaced in Pool program order
                    streams = (
                        ("r", wrw, 0, grp), ("s", wsw, NPAD // 16, gsp),
                    )
                    if tmc[0] % 2 == 1:
                        streams = streams[::-1]
                    tmc[0] += 1
                    for sk, win, stream_off, pool in streams:
                        whi = min(win * WSZ + WSZ, N)
                        wmid = WBASE[win]
                        # Bresenham over combined op sequence -> tmode ops
                        # evenly spaced in Pool program order
                        f0 = int(np.floor(tmc[1] * TMODE_FRAC))
                        f1 = int(np.floor((tmc[1] + 1) * TMODE_FRAC))
                        tmode = f1 > f0
                        tmc[1] += 1
                        icol = stream_off + (goff + c0 * TILE) // 16
                        if tmode:
                            dst = pool.tile([128, B, ni], bf, tag=f"ht{sk}",
                                            name=f"ht{sk}")
                        else:
                            dst = pool.tile([128, cn, B * HID], bf,
                                            tag=f"hp{sk}", name=f"hp{sk}")
                        dview = dst[:]
                        nc.gpsimd.dma_gather(
                            out_ap=dview,
                            in_ap=htab[wmid:whi],
                            idxs_ap=idx_sb[:, icol:icol + ni // 16],
                            num_idxs=ni,
                            num_idxs_reg=ni,
                            elem_size=B * HID,
                            transpose=tmode,
                            single_packet=False,
                            queue_num=0 if tmode else 1 + qc % 3,
                        )
                        if not tmode:
                            qc += 1
                        ops[sk].append((dst, c0, cn, tmode))
                    # ea chunks at half gather-chunk size to save SBUF
                    e0 = c0
                    while e0 < c0 + cn:
                        en = min(c0 + cn - e0, OP_IDX // (2 * TILE))
                        ea_g = eap.tile([ED, en * B * TILE], bf, tag="ea",
                                        name="ea")
                        nc.sync.dma_start(
                            out=ea_g[:],
                            in_=eat_ext[:, B * (goff + e0 * TILE):
                                        B * (goff + (e0 + en) * TILE)])
                        ea_ops.append((ea_g, e0, en, False))
                        e0 += en
                    c0 += cn
                    si += 1

                def _op_slice(oplist, t):
                    for (dst, c0, cn, tm) in oplist:
                        if c0 <= t < c0 + cn:
                            return dst, t - c0, tm
                    raise AssertionError

                # compute loop: quads (2 pairs) of tiles for weight reuse
                npairs = nt // 2
                cpc = [0]  # copy-engine alternation counter

                def prep_rhs(t, rhs, k):
                    """Resolve rhs sources for tile t; emit transposes+copy
                    for plain-gathered streams (copies alternate DVE/ACT)."""
                    plain = []
                    for si_, sk in enumerate(("r", "s")):
                        d, l, tm = _op_slice(ops[sk], t)
                        if tm:
                            rhs[(k, sk)] = d[:, :, l * TILE:(l + 1) * TILE]
                        else:
                            plain.append((si_, sk, d, l))
                    if not plain:
                        return
                    pt = ptp.tile([128, 2, B * HID], bf,
                                  tag="pt", name="pt", space="PSUM")
                    hT = tsb.tile([128, 2, B * HID], bf, tag="hT", name="hT")
                    for (si_, sk, d, l) in plain:
                        for b in range(B):
                            nc.tensor.transpose(
                                out=pt[:, si_, b * HID:(b + 1) * HID],
                                in_=d[:, l, b * HID:(b + 1) * HID],
                                identity=ident[:],
                            )
                    if len(plain) == 2:
                        src = pt[:].bitcast(f32)
                        dst2 = hT[:].bitcast(f32)
                    else:
                        si_ = plain[0][0]
                        src = pt[:, si_, :].bitcast(f32)
                        dst2 = hT[:, si_, :].bitcast(f32)
                    nc.vector.tensor_copy(out=dst2, in_=src)
                    cpc[0] += 1
                    for (si_, sk, d, l) in plain:
                        rhs[(k, sk)] = hT[:, si_, :]

                p = 0
                while p < npairs:
                    np_ = min(1, npairs - p)   # pairs in this weight group
                    y1t = []
                    rhs = {}
                    for j in range(np_):
                        y1 = y1pp.tile([128, 2, B * HID], f32, tag="y1",
                                       name="y1", space="PSUM")
                        y1t.append(y1)
                        for k in range(2):
                            prep_rhs(2 * (p + j) + k, rhs, 2 * j + k)
                    # weight-grouped MLP across the quad (LDW every 4 MMs)
                    for w, skey, start, stop in (
                        (w1r, "r", True, False), (w1s, "s", False, False),
                    ):
                        for j in range(np_):
                            for k in range(2):
                                nc.tensor.matmul(out=y1t[j][:, k, :], lhsT=w[:],
                                                 rhs=rhs[(2 * j + k, skey)],
                                                 start=start, stop=stop)
                    for j in range(np_):
                        for k in range(2):
                            t = 2 * (p + j) + k
                            eg, le, _ = _op_slice(ea_ops, t)
                            nc.tensor.matmul(
                                out=y1t[j][:, k, :], lhsT=w1e[:],
                                rhs=eg[:, le * B * HID:(le + 1) * B * HID],
                                start=False, stop=True)
                    # lagged W2 keeps PE busy during relu
                    for _ in range(np_):
                        flush_w2()
                    for j in range(np_):
                        y1sb = y1sp.tile([128, 2, B * HID], bf, tag="y1sb",
                                         name="y1sb")
                        nc.scalar.activation(out=y1sb[:], in_=y1t[j][:],
                                             func=RELU, bias=b1[:])
                        pend.append({"y1sb": y1sb,
                                     "col0": goff + 2 * (p + j) * TILE,
                                     "npair": 2})
                    p += np_
            while pend:
                flush_w2()
            flush_stg()
    nc.compile()
    return nc


def _run(inputs, trace=False):
    import time as _t
    from concourse.bass_utils import run_bass_kernel_spmd

    htab, wts, cores, meta = _prepare(**inputs)
    key = tuple(meta["S"])
    if key not in _cache:
        t0 = _t.time()
        _cache[key] = _build(meta["S"], meta["NPAD"])
        print(f"[kernel] build+compile: {_t.time()-t0:.1f}s NPAD={meta['NPAD']}")
    nc = _cache[key]

    in_maps = []
    for c in range(NCORES):
        m = {"htab": htab, "eat": cores[c]["eat"], "idx": cores[c]["idx"]}
        m.update({k: wts[k] for k in ("w1r", "w1s", "w1e", "w2", "b1", "ident")})
        in_maps.append(m)

    res = run_bass_kernel_spmd(nc, in_maps, core_ids=list(range(NCORES)),
                               trace=trace)

    y2 = np.zeros((B, EA, 2), dtype=np.float32)
    for c in range(NCORES):
        o = res.results[c]["out"]
        se = cores[c]["slot_edges"]
        valid = se >= 0
        y2[:, se[valid], :] = o[:, :, valid].transpose(1, 2, 0)

    b2 = np.asarray(inputs["b2"], dtype=np.float32)
    out = 0.5 * (y2[:, 0::2, :] + y2[:, 1::2, :]) + b2[None, None, :]
    return out.astype(np.float32), res.exec_time_ns


def kernel(**inputs):
    out, _ = _run(inputs, trace=False)
    return out
